# revision 17
# baseline (speedup 1.0000x reference)
"""Trainium2 Bass kernel for a single-token GQA decoder layer (B=64 batches),
tensor-parallel across 8 NeuronCores.

Contract: kernel(**inputs) takes the FULL fp32 inputs (as produced by the
reference setup_inputs) and returns the FULL [64, 1, 4096] fp32 output.

Sharding (TP-8): core c owns q heads [4c, 4c+4), kv head c, MLP rows
[1792c, 1792(c+1)); hidden dim replicated. One on-device AllReduce (fp8)
after the wo projection; the final down-proj partial sums are reduced on
host.

Perf design v3:
- ALL large inputs ride a pair of HWDGE rings as 81 uniform 1MB slabs
  [128, 8192] fp8: [wqkv x4][K/V x32][wo x2][up/gate x29 bf16-bitcast]
  [down x14 bf16-bitcast]. Phase-1 slabs (wqkv/KV/wo) ride nc.sync only
  (strict FIFO keeps KV sequential); the MLP weight slabs alternate
  sync/scalar so both rings stream during the AllReduce window and the
  MLP phase. Ring depth 16 buffers the collective latency.
- hs is the FIRST DMA; all small constants are packed into ONE [128, 2176]
  image (single DMA) and sliced/bitcast on device — DMA triggers cost
  ~0.7us of sequencer time each and are gated on a ~16-outstanding global
  cap, so fewer+earlier triggers shorten the attention lead-in.
- The RESIDUAL rides the AllReduce in bf16: cc_in = wo_partial*32 + hs*4,
  so the reduced output is hidden*32 directly (bf16 wire: quantizing the
  residual-bearing hidden to fp8 costs ~4e-2 rel err; bf16 is free).
  Post-collective chain is just ar -> PE transpose -> up/gate matmuls;
  rmsnorm-2 runs concurrently on ACT and its scale (with the 1/32 wire
  unscale folded in) is applied after the matmuls via the Silu scale.
- Attention: 3-stage software pipeline (pv runs two groups behind qk) so
  the serialized exp->sums->PV->oT tail hides under later groups' QK/exp.
- Attention path fp8; MLP weights/activations bf16 (fp8 there busts the
  2e-2 budget: the MLP output rms is 3.5x the residual rms).
- Collective in/out + outputs ride gpsimd SWDGE so they never queue
  behind weight slabs on the HWDGE rings.
"""

import numpy as np

import concourse.bass as bass
import concourse.bacc as bacc
import concourse.mybir as mybir
import concourse.tile as tile
from concourse.bass_utils import run_bass_kernel_spmd

FP = mybir.dt.float32
BF = mybir.dt.bfloat16
F8 = mybir.dt.float8e4
AX = mybir.AxisListType
AF = mybir.ActivationFunctionType
ALU = mybir.AluOpType

NCORES = 8
B = 64                    # batch (= tokens, QLEN=1)
DIM = 4096
HD = 128
G = 4                     # local q heads per core
S = 2048                  # prefix length
IL = 14336 // NCORES      # local intermediate = 1792
QKV = (G + 2) * HD        # 768 local qkv rows
EPS = 1e-6
GRP = 4                   # batches per attention group
NGRP = B // GRP           # 16
CW = 448                  # MLP column chunk (IL = 4*448)
EXP_SCALE = 1.0 / float(np.sqrt(HD))
EXP_BIAS = -4.0
WQ_SCALE = 64.0           # host premultiplies wqkv by this (fp8 range)
WO_SCALE = 32.0           # host premultiplies wo by this; the residual
                          # rides the AllReduce at the same x32 wire scale
HS_C = WO_SCALE / NCORES  # per-core residual share on the collective wire

# ---- stream slab indices ----
SLAB_W = 8192             # fp8 bytes per partition per slab
NSLAB_QKV = 4             # slabs 0-3: wqkvT, 8 j-blocks of 768 cols each
SLAB_KV0 = 4              # slabs 4..35: per group t, k at 4+2t, v at 5+2t
SLAB_WO = 36              # slabs 36-37: woT (kk 0,1 | kk 2,3)
SLAB_UG = 38              # slabs 38..66: up/gate bf16, 9 blocks of 448/slab
NSLAB_UG = 29
SLAB_DN = SLAB_UG + NSLAB_UG   # slabs 67..80: down bf16, 8 blocks of 512/slab
NSLAB_DN = 14
NSLAB = SLAB_DN + NSLAB_DN     # 81
RING = 16                 # stream ring depth (SBUF: 16 x 8KB/partition)

# ---- packed constant image byte offsets ([128, CST_W] fp8) ----
CST_W = 2176
C_ID128 = 0               # id128q f8 [128,128] @ 0:128
C_SEL = 128               # sel f8 [128,16] @ 128:144
C_ONES128 = 144           # ones fp32 [128,1] @ 144:148
C_BIASC = 148             # biasc fp32 [128,6] @ 148:172
C_ID64Q = 176             # id64q f8 [64,64] @ 176:240 (parts 0-63)
C_ID64B = 240             # id64b bf16 [64,64] @ 240:368 (parts 0-63)
C_ONES14 = 368            # ones f8 [1,4] @ 368:372 (part 0)
C_QNW = 640               # qnw fp32 [1,128] @ 640:1152 (part 0)
C_KNW = 1152              # knw fp32 [1,128] @ 1152:1664 (part 0)
C_MASK4 = 1664            # mask4 fp32 [4,128] @ 1664:2176 (parts 0-3)


def build_nc():
    nc = bacc.Bacc("TRN2", target_bir_lowering=False, debug=False,
                   num_devices=NCORES)

    # ---- DRAM I/O ----
    strm_d = nc.dram_tensor("strm", [NSLAB, HD, SLAB_W], F8,
                            kind="ExternalInput")
    hs_d = nc.dram_tensor("hs", [B, DIM], FP, kind="ExternalInput")
    cst_d = nc.dram_tensor("cst", [128, CST_W], F8, kind="ExternalInput")

    partial_d = nc.dram_tensor("partial", [B, DIM], BF, kind="ExternalOutput")
    res2_d = nc.dram_tensor("res2", [B, DIM], BF, kind="ExternalOutput")

    with tile.TileContext(nc) as tc:
        with (
            tc.tile_pool(name="const", bufs=1) as constp,
            tc.tile_pool(name="sb", bufs=1) as sb,
            tc.tile_pool(name="strm", bufs=RING) as strmp,
            tc.tile_pool(name="att", bufs=3) as att,
            tc.tile_pool(name="small", bufs=4) as small,
            tc.tile_pool(name="ps_sc", bufs=2, space="PSUM") as ps_sc,
            tc.tile_pool(name="ps_stage", bufs=2, space="PSUM") as ps_stage,
            tc.tile_pool(name="ps_acc", bufs=2, space="PSUM") as ps_acc,
            tc.tile_pool(name="dram", bufs=1, space="DRAM") as dram,
        ):
            # hs is the very first DMA (its consumers gate everything)
            hs = sb.tile([B, DIM], FP, tag="hs")
            nc.scalar.dma_start(hs[:], hs_d[:])
            cst = constp.tile([128, CST_W], F8, tag="cst")
            nc.scalar.dma_start(cst[:], cst_d[:])

            id128q = cst[:, C_ID128:C_ID128 + 128]
            sel = cst[:, C_SEL:C_SEL + 16]
            ones128 = cst[:, C_ONES128:C_ONES128 + 4].bitcast(FP)
            biasc = cst[:, C_BIASC:C_BIASC + 24].bitcast(FP)
            id64q = cst[0:64, C_ID64Q:C_ID64Q + 64]
            id64b = cst[0:64, C_ID64B:C_ID64B + 128].bitcast(BF)
            ones14 = cst[0:1, C_ONES14:C_ONES14 + 4]
            qnw = cst[0:1, C_QNW:C_QNW + 512].bitcast(FP)
            knw = cst[0:1, C_KNW:C_KNW + 512].bitcast(FP)
            mask4 = cst[0:4, C_MASK4:C_MASK4 + 512].bitcast(FP)

            ebias = constp.tile([128, 1], FP, tag="ebias")
            nc.vector.memset(ebias[:], EXP_BIAS)

            # ---- the weight/KV stream: in-order ring over two rings ----
            next_slab = [0]

            def slab(i):
                # Slabs alternate between the SP HWDGE ring and the gpsimd
                # SWDGE path: one HWDGE ring moves data strictly FIFO at
                # ~320GB/s (3.1us/slab), which would pace attention at
                # 6.2us/group; two independent paths restore ~2x. The ACT
                # ring is NOT used — a consumption-gated D2D in the ACT
                # instruction FIFO stalls the exps behind it (measured
                # 11us); the gpsimd queue is idle during attention.
                assert i == next_slab[0], (i, next_slab[0])
                t = strmp.tile([HD, SLAB_W], F8, tag="strm")
                eng = nc.sync if i % 2 == 0 else nc.gpsimd
                eng.dma_start(t[:], strm_d[i])
                next_slab[0] += 1
                return t

            # wqkv slabs lead the stream
            wq_t = [slab(i) for i in range(NSLAB_QKV)]

            # ================= helpers ==================================
            def rmsnorm_rstd(x_sb, tag, c0=1.0 / DIM, c1=EPS):
                """rstd [64,1] fp32 for token-major x_sb [64, DIM];
                sqrt(1/(mean(x^2)+eps)) with optional constant folding."""
                scr = sb.tile([B, DIM], F8, tag="x16")
                ssq = small.tile([B, 1], FP, tag=tag + "ssq")
                nc.scalar.activation(scr[:], x_sb[:], AF.Square,
                                     accum_out=ssq[:])
                t1 = small.tile([B, 1], FP, tag=tag + "t1")
                nc.vector.tensor_scalar(t1[:], ssq[:], c0, c1,
                                        op0=ALU.mult, op1=ALU.add)
                rcp = small.tile([B, 1], FP, tag=tag + "rcp")
                nc.vector.reciprocal(rcp[:], t1[:])
                rstd = small.tile([B, 1], FP, tag=tag + "rstd")
                nc.scalar.activation(rstd[:], rcp[:], AF.Sqrt)
                return rstd

            def transpose_rows(x_sb, ncols, dest, idm):
                """x_sb [64, ncols] -> dest [128, ncols//128*64]."""
                nch = ncols // 128
                for q in range(0, nch, 8):
                    hi = min(nch, q + 8)
                    stage = ps_stage.tile([128, 512], FP, tag="stage")
                    for j in range(q, hi):
                        nc.tensor.matmul(stage[:, (j - q) * 64:(j - q + 1) * 64],
                                         x_sb[:, j * 128:(j + 1) * 128],
                                         idm, start=True, stop=True)
                    nc.vector.tensor_copy(dest[:, q * 64:hi * 64],
                                          stage[:, 0:(hi - q) * 64])

            # ================= RMSNorm 1 + x^T (fp8) ====================
            rstd1 = rmsnorm_rstd(hs, "n1")
            x16 = sb.tile([B, DIM], F8, tag="x16")
            nc.vector.tensor_scalar_mul(x16[:], hs[:], rstd1[:])
            xT = sb.tile([128, B * DIM // 128], F8, tag="xT")   # [128, 2048]
            transpose_rows(x16, DIM, xT, id64q)

            # ================= QKV projection (fp8, x64 scaled) =========
            qkv_a = ps_acc.tile([B, 512], FP, tag="acc")
            qkv_b = ps_acc.tile([B, 256], FP, tag="acc")
            for j in range(32):
                wt = wq_t[j // 8]
                c0 = (j % 8) * QKV
                nc.tensor.matmul(qkv_a[:], xT[:, j * 64:(j + 1) * 64],
                                 wt[:, c0:c0 + 512], start=(j == 0),
                                 stop=(j == 31))
                nc.tensor.matmul(qkv_b[:], xT[:, j * 64:(j + 1) * 64],
                                 wt[:, c0 + 512:c0 + 768], start=(j == 0),
                                 stop=(j == 31))
            qkv_row = sb.tile([B, QKV], BF, tag="qkv_row")
            nc.vector.tensor_scalar_mul(qkv_row[:, 0:512], qkv_a[:],
                                        1.0 / WQ_SCALE)
            nc.vector.tensor_scalar_mul(qkv_row[:, 512:768], qkv_b[:],
                                        1.0 / WQ_SCALE)

            # transpose to [128 hd, 6*64] (fp32) and add bias
            qkvT = sb.tile([128, 6 * 64], FP, tag="qkvT")
            stage6 = ps_stage.tile([128, 512], FP, tag="stage")
            for c in range(6):
                nc.tensor.matmul(stage6[:, c * 64:(c + 1) * 64],
                                 qkv_row[:, c * 128:(c + 1) * 128],
                                 id64b, start=True, stop=True)
            for c in range(6):
                nc.vector.tensor_scalar_add(qkvT[:, c * 64:(c + 1) * 64],
                                            stage6[:, c * 64:(c + 1) * 64],
                                            biasc[:, c:c + 1])

            # ================= q/k rmsnorm (over partition dim HD) ======
            sq2 = sb.tile([128, 320], FP, tag="sq2")
            nc.scalar.activation(sq2[:], qkvT[:, 0:320], AF.Square)
            ss = ps_stage.tile([1, 320], FP, tag="stage")
            nc.tensor.matmul(ss[:], ones128, sq2[:], start=True, stop=True)
            t2 = small.tile([1, 320], FP, tag="t2", bufs=1)
            nc.vector.tensor_scalar(t2[:], ss[:], 1.0 / HD, EPS,
                                    op0=ALU.mult, op1=ALU.add)
            rcp2 = small.tile([1, 320], FP, tag="rcp2", bufs=1)
            nc.vector.reciprocal(rcp2[:], t2[:])
            rstd2 = small.tile([1, 320], FP, tag="rstd2", bufs=1)
            nc.scalar.activation(rstd2[:], rcp2[:], AF.Sqrt)

            bq = ps_stage.tile([128, 256], FP, tag="stage")
            nc.tensor.matmul(bq[:], qnw, rstd2[0:1, 0:256],
                             start=True, stop=True)
            qn = sb.tile([128, 256], F8, tag="qn")
            nc.vector.tensor_tensor(qn[:], qkvT[:, 0:256], bq[:], op=ALU.mult)
            bk = ps_stage.tile([128, 64], FP, tag="stage")
            nc.tensor.matmul(bk[:], knw, rstd2[0:1, 256:320],
                             start=True, stop=True)
            kn = sb.tile([128, 64], F8, tag="kn")
            nc.vector.tensor_tensor(kn[:], qkvT[:, 256:320], bk[:], op=ALU.mult)

            # v_new: v16 [128 hd, 64 tok] fp8, then per-group rows
            # vnewg [4, 16*128]: [b, t*128+d] = v_new[4t+b, d]
            v16 = sb.tile([128, 64], F8, tag="v16")
            nc.vector.tensor_copy(v16[:], qkvT[:, 320:384])
            vnewg = sb.tile([GRP, NGRP * HD], F8, tag="vnewg")
            for t in range(NGRP):
                vg_ps = ps_stage.tile([GRP, HD], FP, tag="stage")
                nc.tensor.matmul(vg_ps[:], v16[:, t * GRP:(t + 1) * GRP],
                                 id128q, start=True, stop=True)
                nc.vector.tensor_copy(vnewg[:, t * HD:(t + 1) * HD], vg_ps[:])

            # q slices ordered [128, tok, g] (col = g*64 + tok)
            qn_r = qn[:].rearrange("p (g t) -> p t g", g=G)

            # ================= attention ================================
            # group t = batches [4t, 4t+4); score rows (b,g) = 32b+g bands.
            # Three-stage software pipeline: pv(t-2) runs under qk(t-1)/
            # qk(t), hiding the exp->sums->PV->oT serial tail.
            oT = sb.tile([128, B * G], F8, tag="oT")   # col = 16t + 4b + g

            def qk_stage(t):
                kt = slab(SLAB_KV0 + 2 * t)
                vt = slab(SLAB_KV0 + 2 * t + 1)

                last = ps_acc.tile([128, 1], FP, tag="acc")
                nc.vector.memset(last[:], 0.0)
                sc_h = []
                for h in range(2):
                    sc = ps_sc.tile([128, 1024], FP, tag="sc")
                    if t == 0:
                        nc.vector.memset(sc[:], 0.0)
                    sc_h.append(sc)
                    for n in range(2):
                        for b in range(GRP):
                            bg = t * GRP + b
                            nc.tensor.matmul(
                                sc[32 * b:32 * b + 4, n * 512:(n + 1) * 512],
                                qn_r[:, bg],
                                kt[:, b * 2048 + (2 * h + n) * 512:
                                   b * 2048 + (2 * h + n + 1) * 512],
                                start=True, stop=True,
                                tile_position=(0, 32 * b))
                    if h == 0:
                        for b in range(GRP):
                            bg = t * GRP + b
                            nc.tensor.matmul(last[32 * b:32 * b + 4, 0:1],
                                             qn_r[:, bg], kn[:, bg:bg + 1],
                                             start=True, stop=True,
                                             tile_position=(0, 32 * b))

                # exp (no max-subtract: scores ~N(0,1); bias keeps fp8 range)
                p_sb = att.tile([128, S], F8, tag="p")
                s1a = small.tile([128, 1], FP, tag="s1a")
                s1b = small.tile([128, 1], FP, tag="s1b")
                nc.scalar.activation(p_sb[:, 0:1024], sc_h[0][:], AF.Exp,
                                     bias=ebias[:], scale=EXP_SCALE,
                                     accum_out=s1a[:])
                nc.scalar.activation(p_sb[:, 1024:2048], sc_h[1][:], AF.Exp,
                                     bias=ebias[:], scale=EXP_SCALE,
                                     accum_out=s1b[:])
                plf = small.tile([128, 1], F8, tag="plf")
                nc.scalar.activation(plf[:], last[:], AF.Exp,
                                     bias=ebias[:], scale=EXP_SCALE)
                return dict(t=t, vt=vt, p_sb=p_sb, s1a=s1a, s1b=s1b, plf=plf)

            o_all = sb.tile([128, NGRP * HD], F8, tag="o_all")

            def pv_pre(cx):
                """Sums + the last-token P4 chain: runs one group after its
                qk so pv_main never stalls the PE FIFO on these DVE hops."""
                plf = cx["plf"]
                stot = small.tile([128, 1], FP, tag="stot")
                nc.vector.tensor_tensor(stot[:], cx["s1a"][:], cx["s1b"][:],
                                        op=ALU.add)
                stot2 = small.tile([128, 1], FP, tag="stot2")
                nc.vector.tensor_tensor(stot2[:], stot[:], plf[:], op=ALU.add)
                rs = small.tile([128, 1], FP, tag="rs")
                nc.vector.reciprocal(rs[:], stot2[:])   # softmax denominator
                cx["rs"] = rs

                # last-token band weights P4 [4, 128] (masked broadcast)
                pl_ps = ps_stage.tile([1, 128], FP, tag="stage")
                nc.tensor.matmul(pl_ps[:], plf[:], id128q,
                                 start=True, stop=True)
                plr = small.tile([1, 128], F8, tag="plr")
                nc.vector.tensor_copy(plr[:], pl_ps[:])
                bc4 = ps_stage.tile([GRP, HD], FP, tag="stage")
                nc.tensor.matmul(bc4[:], ones14, plr[:],
                                 start=True, stop=True)
                p4 = small.tile([GRP, HD], F8, tag="p4")
                nc.vector.tensor_tensor(p4[:], bc4[:], mask4, op=ALU.mult)
                cx["p4"] = p4

            def pv_main(cx):
                t, vt, p_sb = cx["t"], cx["vt"], cx["p_sb"]
                # pT [128 seq, 16 (b,g)] via the selection matrix, in two
                # halves so the PSUM->SBUF copy overlaps the second half
                pTa = att.tile([128, 256], F8, tag="pT")
                for h in range(2):
                    pT_ps = ps_stage.tile([128, 128], FP, tag="stage")
                    for j in range(h * 8, h * 8 + 8):
                        nc.tensor.matmul(pT_ps[:, (j % 8) * 16:
                                               (j % 8 + 1) * 16],
                                         p_sb[:, j * 128:(j + 1) * 128],
                                         sel, start=True, stop=True)
                    nc.vector.tensor_copy(pTa[:, h * 128:(h + 1) * 128],
                                          pT_ps[:])

                # PV band-parallel: o_ps [128 bands, 128 hd]; P4 computed a
                # group ago in pv_pre, so the init never waits on DVE
                o_ps = ps_stage.tile([128, 128], FP, tag="stage")
                nc.tensor.matmul(o_ps[:], cx["p4"][:],
                                 vnewg[:, t * HD:(t + 1) * HD],
                                 start=True, stop=False,
                                 skip_group_check=True)
                for j in range(16):
                    for b in range(GRP):
                        nc.tensor.matmul(
                            o_ps[32 * b:32 * b + 4, :],
                            pTa[:, j * 16 + 4 * b:j * 16 + 4 * b + 4],
                            vt[:, b * 2048 + j * 128:
                               b * 2048 + (j + 1) * 128],
                            start=False, stop=(j == 15),
                            tile_position=(0, 32 * b),
                            skip_group_check=True)
                nc.vector.tensor_scalar_mul(
                    o_all[:, t * HD:(t + 1) * HD], o_ps[:], cx["rs"][:])

            cxs = []
            for t in range(NGRP):
                cxs.append(qk_stage(t))
                if t >= 1:
                    pv_pre(cxs[t - 1])
                if t >= 2:
                    pv_main(cxs[t - 2])
            pv_pre(cxs[NGRP - 1])
            pv_main(cxs[NGRP - 2])
            pv_main(cxs[NGRP - 1])

            # batched oT transposes (PE, overlaps the wo slab DMA)
            for t in range(NGRP):
                oT_ps = ps_stage.tile([128, 128], FP, tag="stage")
                nc.tensor.matmul(oT_ps[:], o_all[:, t * HD:(t + 1) * HD],
                                 id128q, start=True, stop=True)
                oT_v = oT_ps[:].rearrange("p (b x) -> p b x", b=GRP)
                nc.vector.tensor_copy(
                    oT[:, t * 16:(t + 1) * 16].rearrange(
                        "p (b g) -> p b g", b=GRP),
                    oT_v[:, :, 0:G])

            # ======== wo projection (fp8) + residual onto the wire ======
            wo_t = [slab(SLAB_WO), slab(SLAB_WO + 1)]
            oT_r = oT[:].rearrange("p (t b g) -> p g t b", t=NGRP, g=G)
            wo_out = sb.tile([B, DIM], BF, tag="wo_out")
            for n in range(8):
                wo_ps = ps_acc.tile([B, 512], FP, tag="acc")
                for kk in range(4):
                    ws = wo_t[kk // 2]
                    c0 = (kk % 2) * DIM + n * 512
                    nc.tensor.matmul(wo_ps[:], oT_r[:, kk],
                                     ws[:, c0:c0 + 512],
                                     start=(kk == 0), stop=(kk == 3))
                # wire = hs*HS_C + attn_partial*WO_SCALE (wo_ps is already
                # x32: oT is true-scale, woT host-premultiplied by 32)
                nc.vector.scalar_tensor_tensor(
                    wo_out[:, n * 512:(n + 1) * 512],
                    hs[:, n * 512:(n + 1) * 512], HS_C,
                    wo_ps[:], op0=ALU.mult, op1=ALU.add)

            # ================= AllReduce (bf16, x32 scaled) =============
            cc_in = dram.tile([B, DIM], BF)
            cc_out = dram.tile([B, DIM], BF)
            nc.gpsimd.dma_start(cc_in[:], wo_out[:])

            nc.gpsimd.collective_compute(
                "AllReduce", ALU.add,
                replica_groups=[list(range(NCORES))],
                ins=[cc_in[:].opt()], outs=[cc_out[:].opt()],
            )

            # ar = hidden * WO_SCALE (residual already included); reuses
            # wo_out's buffer (free after the cc_in copy)
            ar = sb.tile([B, DIM], BF, tag="wo_out")
            nc.gpsimd.dma_start(ar[:], cc_out[:])

            # hT directly from ar (stays in x32 wire units, bf16)
            hT = sb.tile([128, B * DIM // 128], BF, tag="hT")
            transpose_rows(ar, DIM, hT, id64b)

            # hidden (true units) for res2 + rmsnorm; off the critical path
            hidden = sb.tile([B, DIM], BF, tag="hid")
            nc.vector.tensor_scalar_mul(hidden[:], ar[:], 1.0 / WO_SCALE)
            nc.gpsimd.dma_start(res2_d[:], hidden[:])

            # ========== RMSNorm 2 (deferred, wire-unscale folded) =======
            # rstd_wire = rstd_true/WO_SCALE: fold via
            # t1 = ssq*(WO_SCALE^2/DIM) + WO_SCALE^2*eps
            rstd2h = rmsnorm_rstd(hidden, "n2",
                                  c0=WO_SCALE * WO_SCALE / DIM,
                                  c1=WO_SCALE * WO_SCALE * EPS)

            ug_slabs = {}

            def ug_block(idx):
                s = SLAB_UG + idx // 9
                if s not in ug_slabs:
                    ug_slabs[s] = slab(s)
                return ug_slabs[s], (idx % 9) * CW

            g_row = sb.tile([B, IL], BF, tag="g_row")
            gu_row = sb.tile([B, IL], BF, tag="gu_row")
            for c in range(4):
                up_ps = ps_acc.tile([B, CW], FP, tag="acc")
                gt_ps = ps_acc.tile([B, CW], FP, tag="acc")
                for j in range(32):
                    su, cu = ug_block(c * 64 + j * 2)
                    sg, cg = ug_block(c * 64 + j * 2 + 1)
                    nc.tensor.matmul(up_ps[:],
                                     hT[:, j * 64:(j + 1) * 64],
                                     su[:].bitcast(BF)[:, cu:cu + CW],
                                     start=(j == 0), stop=(j == 31))
                    nc.tensor.matmul(gt_ps[:],
                                     hT[:, j * 64:(j + 1) * 64],
                                     sg[:].bitcast(BF)[:, cg:cg + CW],
                                     start=(j == 0), stop=(j == 31))
                # g = silu(rstd_w * gate_wire); gu = (up_wire*rstd_w) * g
                nc.scalar.activation(g_row[:, c * CW:(c + 1) * CW], gt_ps[:],
                                     AF.Silu, scale=rstd2h[:])
                nc.vector.scalar_tensor_tensor(
                    gu_row[:, c * CW:(c + 1) * CW], up_ps[:], rstd2h[:],
                    g_row[:, c * CW:(c + 1) * CW],
                    op0=ALU.mult, op1=ALU.mult)

            guT = sb.tile([128, 14 * 64], BF, tag="guT")
            transpose_rows(gu_row, IL, guT, id64b)

            dn_slabs = {}

            def dn_block(idx):
                s = SLAB_DN + idx // 8
                if s not in dn_slabs:
                    dn_slabs[s] = slab(s)
                return dn_slabs[s], (idx % 8) * 512

            for n in range(8):
                dn_ps = ps_acc.tile([B, 512], FP, tag="acc")
                for cc in range(14):
                    sd, col = dn_block(n * 14 + cc)
                    nc.tensor.matmul(dn_ps[:], guT[:, cc * 64:(cc + 1) * 64],
                                     sd[:].bitcast(BF)[:, col:col + 512],
                                     start=(cc == 0), stop=(cc == 13))
                stg = small.tile([B, 512], BF, tag="ostg", bufs=2)
                nc.vector.tensor_copy(stg[:], dn_ps[:])
                nc.gpsimd.dma_start(partial_d[:, n * 512:(n + 1) * 512],
                                    stg[:])

            assert next_slab[0] == NSLAB, next_slab[0]

    nc.compile()
    return nc


def shard_inputs(inputs):
    """Full fp32 inputs -> list of 8 per-core input maps (host prep)."""
    f32 = np.float32
    bf16 = mybir.dt.np(BF)
    f8 = mybir.dt.np(F8)
    hs = np.ascontiguousarray(inputs["hidden_states"].reshape(B, DIM), f32)
    wqkv = np.asarray(inputs["wqkv_w"], f32)
    wb = np.asarray(inputs["wqkv_b"], f32)
    wo = np.asarray(inputs["wo_w"], f32)
    up = np.asarray(inputs["up_w"], f32)
    gate = np.asarray(inputs["gate_w"], f32)
    down = np.asarray(inputs["down_w"], f32)
    qnorm = np.asarray(inputs["qnorm_w"], f32)
    knorm = np.asarray(inputs["knorm_w"], f32)
    iln = np.asarray(inputs["in_ln_w"], f32)
    pln = np.asarray(inputs["post_ln_w"], f32)
    kc = np.asarray(inputs["k_cache"], f32)   # [B, S, 8, HD]
    vc = np.asarray(inputs["v_cache"], f32)

    # selection matrix [128 bands, 16]: SEL[32b+g, 4b+g] = 1
    sel = np.zeros((HD, 16), f8)
    mask4 = np.zeros((GRP, HD), f32)
    for b in range(GRP):
        for g in range(G):
            sel[32 * b + g, 4 * b + g] = 1.0
            mask4[b, 32 * b + g] = 1.0

    H = 32
    maps = []
    for c in range(NCORES):
        strm = np.zeros((NSLAB, HD, SLAB_W), f8)

        # --- slabs 0-3: wqkvT images (8 j-blocks of 768 cols each) ---
        wq = wqkv[c * G * HD:(c + 1) * G * HD]              # [512, DIM]
        wk = wqkv[H * HD + c * HD:H * HD + (c + 1) * HD]    # [128, DIM]
        wv = wqkv[(H + 8) * HD + c * HD:(H + 8) * HD + (c + 1) * HD]
        wloc = np.concatenate([wq, wk, wv], axis=0)         # [768, DIM]
        wqkvT = (wloc * iln[None, :] * WQ_SCALE).T.astype(f8)  # [DIM, 768]
        strm[0:4, :, 0:8 * QKV] = (
            wqkvT.reshape(4, 8, HD, QKV).transpose(0, 2, 1, 3)
            .reshape(4, HD, 8 * QKV))

        bq = wb[c * G * HD:(c + 1) * G * HD]
        bk = wb[H * HD + c * HD:H * HD + (c + 1) * HD]
        bv = wb[(H + 8) * HD + c * HD:(H + 8) * HD + (c + 1) * HD]
        biasc = np.ascontiguousarray(
            np.concatenate([bq, bk, bv]).reshape(6, HD).T)  # [128, 6]

        # --- packed constant image ---
        cst8 = np.zeros((128, CST_W), np.uint8)
        cst8[:, C_ID128:C_ID128 + 128] = np.eye(128, dtype=f8).view(np.uint8)
        cst8[:, C_SEL:C_SEL + 16] = sel.view(np.uint8)
        cst8[:, C_ONES128:C_ONES128 + 4] = \
            np.ones((HD, 1), f32).view(np.uint8)
        cst8[:, C_BIASC:C_BIASC + 24] = biasc.astype(f32).view(np.uint8)
        cst8[0:64, C_ID64Q:C_ID64Q + 64] = \
            np.eye(64, dtype=f8).view(np.uint8)
        cst8[0:64, C_ID64B:C_ID64B + 128] = \
            np.eye(64, dtype=bf16).view(np.uint8)
        cst8[0:1, C_ONES14:C_ONES14 + 4] = \
            np.ones((1, GRP), f8).view(np.uint8)
        cst8[0:1, C_QNW:C_QNW + 512] = \
            qnorm.reshape(1, HD).astype(f32).view(np.uint8)
        cst8[0:1, C_KNW:C_KNW + 512] = \
            knorm.reshape(1, HD).astype(f32).view(np.uint8)
        cst8[0:4, C_MASK4:C_MASK4 + 512] = mask4.view(np.uint8)
        cst = cst8.view(f8)

        # --- slabs 4..35: KV (k seq-transposed, v seq-major) ---
        kT = kc[:, :, c, :].transpose(0, 2, 1).astype(f8)   # [B, HD, S]
        vsm = (vc[:, :, c, :].reshape(B, 16, 128, HD)
               .transpose(0, 2, 1, 3).reshape(B, HD, S).astype(f8))
        for t in range(NGRP):
            for b in range(GRP):
                strm[SLAB_KV0 + 2 * t, :, b * S:(b + 1) * S] = kT[t * GRP + b]
                strm[SLAB_KV0 + 2 * t + 1, :, b * S:(b + 1) * S] = \
                    vsm[t * GRP + b]

        # --- slabs 36-37: woT ---
        woT = (wo[:, c * G * HD:(c + 1) * G * HD].T * WO_SCALE).astype(f8)
        wo_img = (woT.reshape(4, HD, DIM).transpose(1, 0, 2)
                  .reshape(HD, 4 * DIM))
        strm[SLAB_WO] = wo_img[:, 0:SLAB_W]
        strm[SLAB_WO + 1] = wo_img[:, SLAB_W:2 * SLAB_W]

        # --- slabs 38..66: up/gate bf16, block id = c*64+j*2+{0=up,1=gate},
        #     9 blocks of 448 bf16 cols per slab ---
        upT = ((up[c * IL:(c + 1) * IL] * pln[None, :]).T).astype(bf16)
        gateT = ((gate[c * IL:(c + 1) * IL] * pln[None, :]).T).astype(bf16)
        ug_bf = np.zeros((NSLAB_UG, HD, SLAB_W // 2), bf16)
        for cch in range(4):
            for j in range(32):
                for g, wT in ((0, upT), (1, gateT)):
                    idx = cch * 64 + j * 2 + g
                    s_, b_ = idx // 9, (idx % 9) * CW
                    ug_bf[s_, :, b_:b_ + CW] = \
                        wT[j * HD:(j + 1) * HD, cch * CW:(cch + 1) * CW]
        strm[SLAB_UG:SLAB_UG + NSLAB_UG] = \
            ug_bf.view(np.uint8).reshape(NSLAB_UG, HD, SLAB_W).view(f8)

        # --- slabs 67..80: down bf16, block id = n*14+cc, 8 of 512/slab ---
        downT = down[:, c * IL:(c + 1) * IL].T.astype(bf16)  # [IL, DIM]
        dn_bf = np.zeros((NSLAB_DN, HD, SLAB_W // 2), bf16)
        for n in range(8):
            for cc in range(14):
                idx = n * 14 + cc
                s_, b_ = idx // 8, (idx % 8) * 512
                dn_bf[s_, :, b_:b_ + 512] = \
                    downT[cc * HD:(cc + 1) * HD, n * 512:(n + 1) * 512]
        strm[SLAB_DN:SLAB_DN + NSLAB_DN] = \
            dn_bf.view(np.uint8).reshape(NSLAB_DN, HD, SLAB_W).view(f8)

        maps.append({"strm": strm, "hs": hs, "cst": cst})
    return maps


_NC = None


def _get_nc():
    global _NC
    if _NC is None:
        _NC = build_nc()
    return _NC


def run(inputs, **kw):
    nc = _get_nc()
    in_maps = shard_inputs(inputs)
    res = run_bass_kernel_spmd(nc, in_maps, list(range(NCORES)), **kw)
    out = res.results[0]["res2"].astype(np.float64)
    for c in range(NCORES):
        out = out + res.results[c]["partial"].astype(np.float64)
    return out.astype(np.float32).reshape(B, 1, DIM), res


def kernel(**inputs):
    out, _ = run(inputs)
    return out


# revision 18
# speedup vs baseline: 1.1588x; 1.1588x over previous
"""Trainium2 Bass kernel for a single-token GQA decoder layer (B=64 batches),
tensor-parallel across 8 NeuronCores.

Contract: kernel(**inputs) takes the FULL fp32 inputs (as produced by the
reference setup_inputs) and returns the FULL [64, 1, 4096] fp32 output.

Sharding (TP-8): core c owns q heads [4c, 4c+4), kv head c, MLP rows
[1792c, 1792(c+1)); hidden dim replicated. One on-device AllReduce (fp8)
after the wo projection; the final down-proj partial sums are reduced on
host.

Perf design v3:
- ALL large inputs ride a pair of HWDGE rings as 81 uniform 1MB slabs
  [128, 8192] fp8: [wqkv x4][K/V x32][wo x2][up/gate x29 bf16-bitcast]
  [down x14 bf16-bitcast]. Phase-1 slabs (wqkv/KV/wo) ride nc.sync only
  (strict FIFO keeps KV sequential); the MLP weight slabs alternate
  sync/scalar so both rings stream during the AllReduce window and the
  MLP phase. Ring depth 16 buffers the collective latency.
- hs is the FIRST DMA; all small constants are packed into ONE [128, 2176]
  image (single DMA) and sliced/bitcast on device — DMA triggers cost
  ~0.7us of sequencer time each and are gated on a ~16-outstanding global
  cap, so fewer+earlier triggers shorten the attention lead-in.
- The RESIDUAL rides the AllReduce in bf16: cc_in = wo_partial*32 + hs*4,
  so the reduced output is hidden*32 directly (bf16 wire: quantizing the
  residual-bearing hidden to fp8 costs ~4e-2 rel err; bf16 is free).
  Post-collective chain is just ar -> PE transpose -> up/gate matmuls;
  rmsnorm-2 runs concurrently on ACT and its scale (with the 1/32 wire
  unscale folded in) is applied after the matmuls via the Silu scale.
- Attention: 3-stage software pipeline (pv runs two groups behind qk) so
  the serialized exp->sums->PV->oT tail hides under later groups' QK/exp.
- Attention path fp8; MLP weights/activations bf16 (fp8 there busts the
  2e-2 budget: the MLP output rms is 3.5x the residual rms).
- Collective in/out + outputs ride gpsimd SWDGE so they never queue
  behind weight slabs on the HWDGE rings.
"""

import numpy as np

import concourse.bass as bass
import concourse.bacc as bacc
import concourse.mybir as mybir
import concourse.tile as tile
from concourse.bass_utils import run_bass_kernel_spmd

FP = mybir.dt.float32
BF = mybir.dt.bfloat16
F8 = mybir.dt.float8e4
AX = mybir.AxisListType
AF = mybir.ActivationFunctionType
ALU = mybir.AluOpType

NCORES = 8
B = 64                    # batch (= tokens, QLEN=1)
DIM = 4096
HD = 128
G = 4                     # local q heads per core
S = 2048                  # prefix length
IL = 14336 // NCORES      # local intermediate = 1792
QKV = (G + 2) * HD        # 768 local qkv rows
EPS = 1e-6
GRP = 4                   # batches per attention group
NGRP = B // GRP           # 16
CW = 448                  # MLP column chunk (IL = 4*448)
EXP_SCALE = 1.0 / float(np.sqrt(HD))
EXP_BIAS = -4.0
WQ_SCALE = 64.0           # host premultiplies wqkv by this (fp8 range)
WO_SCALE = 32.0           # host premultiplies wo by this; the residual
                          # rides the AllReduce at the same x32 wire scale
HS_C = WO_SCALE / NCORES  # per-core residual share on the collective wire

# ---- stream slab indices ----
SLAB_W = 8192             # fp8 bytes per partition per slab
NSLAB_QKV = 4             # slabs 0-3: wqkvT, 8 j-blocks of 768 cols each
SLAB_KV0 = 4              # slabs 4..35: per group t, k at 4+2t, v at 5+2t
SLAB_WO = 36              # slabs 36-37: woT (kk 0,1 | kk 2,3)
SLAB_UG = 38              # slabs 38..66: up/gate bf16, 9 blocks of 448/slab
NSLAB_UG = 29
SLAB_DN = SLAB_UG + NSLAB_UG   # slabs 67..80: down bf16, 8 blocks of 512/slab
NSLAB_DN = 14
NSLAB = SLAB_DN + NSLAB_DN     # 81
RING = 16                 # stream ring depth (SBUF: 16 x 8KB/partition)

# ---- packed constant image byte offsets ([128, CST_W] fp8) ----
CST_W = 2176
C_ID128 = 0               # id128q f8 [128,128] @ 0:128
C_SEL = 128               # sel f8 [128,16] @ 128:144
C_ONES128 = 144           # ones fp32 [128,1] @ 144:148
C_BIASC = 148             # biasc fp32 [128,6] @ 148:172
C_ID64Q = 176             # id64q f8 [64,64] @ 176:240 (parts 0-63)
C_ID64B = 240             # id64b bf16 [64,64] @ 240:368 (parts 0-63)
C_ONES14 = 368            # ones f8 [1,4] @ 368:372 (part 0)
C_QNW = 640               # qnw fp32 [1,128] @ 640:1152 (part 0)
C_KNW = 1152              # knw fp32 [1,128] @ 1152:1664 (part 0)
C_MASK4 = 1664            # mask4 fp32 [4,128] @ 1664:2176 (parts 0-3)


def build_nc():
    nc = bacc.Bacc("TRN2", target_bir_lowering=False, debug=False,
                   num_devices=NCORES)

    # ---- DRAM I/O ----
    strm_d = nc.dram_tensor("strm", [NSLAB, HD, SLAB_W], F8,
                            kind="ExternalInput")
    hs_d = nc.dram_tensor("hs", [B, DIM], FP, kind="ExternalInput")
    cst_d = nc.dram_tensor("cst", [128, CST_W], F8, kind="ExternalInput")

    partial_d = nc.dram_tensor("partial", [B, DIM], BF, kind="ExternalOutput")
    res2_d = nc.dram_tensor("res2", [B, DIM], BF, kind="ExternalOutput")

    with tile.TileContext(nc) as tc:
        with (
            tc.tile_pool(name="const", bufs=1) as constp,
            tc.tile_pool(name="sb", bufs=1) as sb,
            tc.tile_pool(name="strm", bufs=RING) as strmp,
            tc.tile_pool(name="att", bufs=3) as att,
            tc.tile_pool(name="small", bufs=4) as small,
            tc.tile_pool(name="ps_sc", bufs=2, space="PSUM") as ps_sc,
            tc.tile_pool(name="ps_stage", bufs=2, space="PSUM") as ps_stage,
            tc.tile_pool(name="ps_acc", bufs=2, space="PSUM") as ps_acc,
            tc.tile_pool(name="dram", bufs=1, space="DRAM") as dram,
        ):
            # hs is the very first DMA (its consumers gate everything)
            hs = sb.tile([B, DIM], FP, tag="hs")
            nc.scalar.dma_start(hs[:], hs_d[:])
            cst = constp.tile([128, CST_W], F8, tag="cst")
            nc.scalar.dma_start(cst[:], cst_d[:])

            id128q = cst[:, C_ID128:C_ID128 + 128]
            sel = cst[:, C_SEL:C_SEL + 16]
            ones128 = cst[:, C_ONES128:C_ONES128 + 4].bitcast(FP)
            biasc = cst[:, C_BIASC:C_BIASC + 24].bitcast(FP)
            id64q = cst[0:64, C_ID64Q:C_ID64Q + 64]
            id64b = cst[0:64, C_ID64B:C_ID64B + 128].bitcast(BF)
            ones14 = cst[0:1, C_ONES14:C_ONES14 + 4]
            qnw = cst[0:1, C_QNW:C_QNW + 512].bitcast(FP)
            knw = cst[0:1, C_KNW:C_KNW + 512].bitcast(FP)
            mask4 = cst[0:4, C_MASK4:C_MASK4 + 512].bitcast(FP)

            ebias = constp.tile([128, 1], FP, tag="ebias")
            nc.vector.memset(ebias[:], EXP_BIAS)

            # ---- the weight/KV stream: in-order ring over two rings ----
            next_slab = [0]

            def slab(i):
                # One HWDGE ring moves data strictly FIFO at ~320GB/s
                # (3.1us per 1MB slab), which alone would pace attention at
                # 6.2us/group (2 slabs per group). So phase-1 splits across
                # BOTH HWDGE rings: K slabs on SP, V slabs on ACT. A D2D on
                # the ACT ring is safe ONLY if its gating wait releases
                # before the exp stream reaches its FIFO slot — true for V
                # slabs (ring wait = attention progress 8 groups back) but
                # NOT for MLP weight slabs (gated on post-collective
                # consumption, measured 11us exp stalls) — those all ride
                # SP. (SWDGE bulk transfers measured slower; not used.)
                assert i == next_slab[0], (i, next_slab[0])
                t = strmp.tile([HD, SLAB_W], F8, tag="strm")
                eng = nc.scalar if (i < SLAB_WO and i % 2 == 1) else nc.sync
                eng.dma_start(t[:], strm_d[i])
                next_slab[0] += 1
                return t

            # wqkv slabs lead the stream
            wq_t = [slab(i) for i in range(NSLAB_QKV)]

            # ================= helpers ==================================
            def rmsnorm_rstd(x_sb, tag, c0=1.0 / DIM, c1=EPS):
                """rstd [64,1] fp32 for token-major x_sb [64, DIM];
                sqrt(1/(mean(x^2)+eps)) with optional constant folding."""
                scr = sb.tile([B, DIM], F8, tag="x16")
                ssq = small.tile([B, 1], FP, tag=tag + "ssq")
                nc.scalar.activation(scr[:], x_sb[:], AF.Square,
                                     accum_out=ssq[:])
                t1 = small.tile([B, 1], FP, tag=tag + "t1")
                nc.vector.tensor_scalar(t1[:], ssq[:], c0, c1,
                                        op0=ALU.mult, op1=ALU.add)
                rcp = small.tile([B, 1], FP, tag=tag + "rcp")
                nc.vector.reciprocal(rcp[:], t1[:])
                rstd = small.tile([B, 1], FP, tag=tag + "rstd")
                nc.scalar.activation(rstd[:], rcp[:], AF.Sqrt)
                return rstd

            def transpose_rows(x_sb, ncols, dest, idm):
                """x_sb [64, ncols] -> dest [128, ncols//128*64]."""
                nch = ncols // 128
                for q in range(0, nch, 8):
                    hi = min(nch, q + 8)
                    stage = ps_stage.tile([128, 512], FP, tag="stage")
                    for j in range(q, hi):
                        nc.tensor.matmul(stage[:, (j - q) * 64:(j - q + 1) * 64],
                                         x_sb[:, j * 128:(j + 1) * 128],
                                         idm, start=True, stop=True)
                    nc.vector.tensor_copy(dest[:, q * 64:hi * 64],
                                          stage[:, 0:(hi - q) * 64])

            # ================= RMSNorm 1 + x^T (fp8) ====================
            rstd1 = rmsnorm_rstd(hs, "n1")
            x16 = sb.tile([B, DIM], F8, tag="x16")
            nc.vector.tensor_scalar_mul(x16[:], hs[:], rstd1[:])
            xT = sb.tile([128, B * DIM // 128], F8, tag="xT")   # [128, 2048]
            transpose_rows(x16, DIM, xT, id64q)

            # ================= QKV projection (fp8, x64 scaled) =========
            qkv_a = ps_acc.tile([B, 512], FP, tag="acc")
            qkv_b = ps_acc.tile([B, 256], FP, tag="acc")
            for j in range(32):
                wt = wq_t[j // 8]
                c0 = (j % 8) * QKV
                nc.tensor.matmul(qkv_a[:], xT[:, j * 64:(j + 1) * 64],
                                 wt[:, c0:c0 + 512], start=(j == 0),
                                 stop=(j == 31))
                nc.tensor.matmul(qkv_b[:], xT[:, j * 64:(j + 1) * 64],
                                 wt[:, c0 + 512:c0 + 768], start=(j == 0),
                                 stop=(j == 31))
            qkv_row = sb.tile([B, QKV], BF, tag="qkv_row")
            nc.vector.tensor_scalar_mul(qkv_row[:, 0:512], qkv_a[:],
                                        1.0 / WQ_SCALE)
            nc.vector.tensor_scalar_mul(qkv_row[:, 512:768], qkv_b[:],
                                        1.0 / WQ_SCALE)

            # transpose to [128 hd, 6*64] (fp32) and add bias
            qkvT = sb.tile([128, 6 * 64], FP, tag="qkvT")
            stage6 = ps_stage.tile([128, 512], FP, tag="stage")
            for c in range(6):
                nc.tensor.matmul(stage6[:, c * 64:(c + 1) * 64],
                                 qkv_row[:, c * 128:(c + 1) * 128],
                                 id64b, start=True, stop=True)
            for c in range(6):
                nc.vector.tensor_scalar_add(qkvT[:, c * 64:(c + 1) * 64],
                                            stage6[:, c * 64:(c + 1) * 64],
                                            biasc[:, c:c + 1])

            # ================= q/k rmsnorm (over partition dim HD) ======
            sq2 = sb.tile([128, 320], FP, tag="sq2")
            nc.scalar.activation(sq2[:], qkvT[:, 0:320], AF.Square)
            ss = ps_stage.tile([1, 320], FP, tag="stage")
            nc.tensor.matmul(ss[:], ones128, sq2[:], start=True, stop=True)
            t2 = small.tile([1, 320], FP, tag="t2", bufs=1)
            nc.vector.tensor_scalar(t2[:], ss[:], 1.0 / HD, EPS,
                                    op0=ALU.mult, op1=ALU.add)
            rcp2 = small.tile([1, 320], FP, tag="rcp2", bufs=1)
            nc.vector.reciprocal(rcp2[:], t2[:])
            rstd2 = small.tile([1, 320], FP, tag="rstd2", bufs=1)
            nc.scalar.activation(rstd2[:], rcp2[:], AF.Sqrt)

            bq = ps_stage.tile([128, 256], FP, tag="stage")
            nc.tensor.matmul(bq[:], qnw, rstd2[0:1, 0:256],
                             start=True, stop=True)
            qn = sb.tile([128, 256], F8, tag="qn")
            nc.vector.tensor_tensor(qn[:], qkvT[:, 0:256], bq[:], op=ALU.mult)
            bk = ps_stage.tile([128, 64], FP, tag="stage")
            nc.tensor.matmul(bk[:], knw, rstd2[0:1, 256:320],
                             start=True, stop=True)
            kn = sb.tile([128, 64], F8, tag="kn")
            nc.vector.tensor_tensor(kn[:], qkvT[:, 256:320], bk[:], op=ALU.mult)

            # v_new: v16 [128 hd, 64 tok] fp8, then per-group rows
            # vnewg [4, 16*128]: [b, t*128+d] = v_new[4t+b, d]
            v16 = sb.tile([128, 64], F8, tag="v16")
            nc.vector.tensor_copy(v16[:], qkvT[:, 320:384])
            vnewg = sb.tile([GRP, NGRP * HD], F8, tag="vnewg")
            for t in range(NGRP):
                vg_ps = ps_stage.tile([GRP, HD], FP, tag="stage")
                nc.tensor.matmul(vg_ps[:], v16[:, t * GRP:(t + 1) * GRP],
                                 id128q, start=True, stop=True)
                nc.vector.tensor_copy(vnewg[:, t * HD:(t + 1) * HD], vg_ps[:])

            # q slices ordered [128, tok, g] (col = g*64 + tok)
            qn_r = qn[:].rearrange("p (g t) -> p t g", g=G)

            # ================= attention ================================
            # group t = batches [4t, 4t+4); score rows (b,g) = 32b+g bands.
            # Three-stage software pipeline: pv(t-2) runs under qk(t-1)/
            # qk(t), hiding the exp->sums->PV->oT serial tail.
            oT = sb.tile([128, B * G], F8, tag="oT")   # col = 16t + 4b + g

            def qk_stage(t):
                kt = slab(SLAB_KV0 + 2 * t)
                vt = slab(SLAB_KV0 + 2 * t + 1)

                last = ps_acc.tile([128, 1], FP, tag="acc")
                nc.vector.memset(last[:], 0.0)
                sc_h = []
                for h in range(2):
                    sc = ps_sc.tile([128, 1024], FP, tag="sc")
                    if t == 0:
                        nc.vector.memset(sc[:], 0.0)
                    sc_h.append(sc)
                    for n in range(2):
                        for b in range(GRP):
                            bg = t * GRP + b
                            nc.tensor.matmul(
                                sc[32 * b:32 * b + 4, n * 512:(n + 1) * 512],
                                qn_r[:, bg],
                                kt[:, b * 2048 + (2 * h + n) * 512:
                                   b * 2048 + (2 * h + n + 1) * 512],
                                start=True, stop=True,
                                tile_position=(0, 32 * b))
                    if h == 0:
                        for b in range(GRP):
                            bg = t * GRP + b
                            nc.tensor.matmul(last[32 * b:32 * b + 4, 0:1],
                                             qn_r[:, bg], kn[:, bg:bg + 1],
                                             start=True, stop=True,
                                             tile_position=(0, 32 * b))

                # exp (no max-subtract: scores ~N(0,1); bias keeps fp8 range)
                p_sb = att.tile([128, S], F8, tag="p")
                s1a = small.tile([128, 1], FP, tag="s1a")
                s1b = small.tile([128, 1], FP, tag="s1b")
                nc.scalar.activation(p_sb[:, 0:1024], sc_h[0][:], AF.Exp,
                                     bias=ebias[:], scale=EXP_SCALE,
                                     accum_out=s1a[:])
                nc.scalar.activation(p_sb[:, 1024:2048], sc_h[1][:], AF.Exp,
                                     bias=ebias[:], scale=EXP_SCALE,
                                     accum_out=s1b[:])
                plf = small.tile([128, 1], F8, tag="plf")
                nc.scalar.activation(plf[:], last[:], AF.Exp,
                                     bias=ebias[:], scale=EXP_SCALE)
                return dict(t=t, vt=vt, p_sb=p_sb, s1a=s1a, s1b=s1b, plf=plf)

            o_all = sb.tile([128, NGRP * HD], F8, tag="o_all")

            def pv_pre(cx):
                """Sums + the last-token P4 chain: runs one group after its
                qk so pv_main never stalls the PE FIFO on these DVE hops."""
                plf = cx["plf"]
                stot = small.tile([128, 1], FP, tag="stot")
                nc.vector.tensor_tensor(stot[:], cx["s1a"][:], cx["s1b"][:],
                                        op=ALU.add)
                stot2 = small.tile([128, 1], FP, tag="stot2")
                nc.vector.tensor_tensor(stot2[:], stot[:], plf[:], op=ALU.add)
                rs = small.tile([128, 1], FP, tag="rs")
                nc.vector.reciprocal(rs[:], stot2[:])   # softmax denominator
                cx["rs"] = rs

                # last-token band weights P4 [4, 128] (masked broadcast)
                pl_ps = ps_stage.tile([1, 128], FP, tag="stage")
                nc.tensor.matmul(pl_ps[:], plf[:], id128q,
                                 start=True, stop=True)
                plr = small.tile([1, 128], F8, tag="plr")
                nc.vector.tensor_copy(plr[:], pl_ps[:])
                bc4 = ps_stage.tile([GRP, HD], FP, tag="stage")
                nc.tensor.matmul(bc4[:], ones14, plr[:],
                                 start=True, stop=True)
                p4 = small.tile([GRP, HD], F8, tag="p4")
                nc.vector.tensor_tensor(p4[:], bc4[:], mask4, op=ALU.mult)
                cx["p4"] = p4

            def pv_main(cx):
                t, vt, p_sb = cx["t"], cx["vt"], cx["p_sb"]
                # pT [128 seq, 16 (b,g)] via the selection matrix, in two
                # halves so the PSUM->SBUF copy overlaps the second half
                pTa = att.tile([128, 256], F8, tag="pT")
                for h in range(2):
                    pT_ps = ps_stage.tile([128, 128], FP, tag="stage")
                    for j in range(h * 8, h * 8 + 8):
                        nc.tensor.matmul(pT_ps[:, (j % 8) * 16:
                                               (j % 8 + 1) * 16],
                                         p_sb[:, j * 128:(j + 1) * 128],
                                         sel, start=True, stop=True)
                    nc.vector.tensor_copy(pTa[:, h * 128:(h + 1) * 128],
                                          pT_ps[:])

                # PV band-parallel: o_ps [128 bands, 128 hd]; P4 computed a
                # group ago in pv_pre, so the init never waits on DVE
                o_ps = ps_stage.tile([128, 128], FP, tag="stage")
                nc.tensor.matmul(o_ps[:], cx["p4"][:],
                                 vnewg[:, t * HD:(t + 1) * HD],
                                 start=True, stop=False,
                                 skip_group_check=True)
                for j in range(16):
                    for b in range(GRP):
                        nc.tensor.matmul(
                            o_ps[32 * b:32 * b + 4, :],
                            pTa[:, j * 16 + 4 * b:j * 16 + 4 * b + 4],
                            vt[:, b * 2048 + j * 128:
                               b * 2048 + (j + 1) * 128],
                            start=False, stop=(j == 15),
                            tile_position=(0, 32 * b),
                            skip_group_check=True)
                nc.vector.tensor_scalar_mul(
                    o_all[:, t * HD:(t + 1) * HD], o_ps[:], cx["rs"][:])

            cxs = []
            for t in range(NGRP):
                cxs.append(qk_stage(t))
                if t >= 1:
                    pv_pre(cxs[t - 1])
                if t >= 2:
                    pv_main(cxs[t - 2])
            pv_pre(cxs[NGRP - 1])
            pv_main(cxs[NGRP - 2])
            pv_main(cxs[NGRP - 1])

            # batched oT transposes (PE, overlaps the wo slab DMA)
            for t in range(NGRP):
                oT_ps = ps_stage.tile([128, 128], FP, tag="stage")
                nc.tensor.matmul(oT_ps[:], o_all[:, t * HD:(t + 1) * HD],
                                 id128q, start=True, stop=True)
                oT_v = oT_ps[:].rearrange("p (b x) -> p b x", b=GRP)
                nc.vector.tensor_copy(
                    oT[:, t * 16:(t + 1) * 16].rearrange(
                        "p (b g) -> p b g", b=GRP),
                    oT_v[:, :, 0:G])

            # ======== wo projection (fp8) + residual onto the wire ======
            wo_t = [slab(SLAB_WO), slab(SLAB_WO + 1)]
            oT_r = oT[:].rearrange("p (t b g) -> p g t b", t=NGRP, g=G)
            wo_out = sb.tile([B, DIM], BF, tag="wo_out")
            for n in range(8):
                wo_ps = ps_acc.tile([B, 512], FP, tag="acc")
                for kk in range(4):
                    ws = wo_t[kk // 2]
                    c0 = (kk % 2) * DIM + n * 512
                    nc.tensor.matmul(wo_ps[:], oT_r[:, kk],
                                     ws[:, c0:c0 + 512],
                                     start=(kk == 0), stop=(kk == 3))
                # wire = hs*HS_C + attn_partial*WO_SCALE (wo_ps is already
                # x32: oT is true-scale, woT host-premultiplied by 32)
                nc.vector.scalar_tensor_tensor(
                    wo_out[:, n * 512:(n + 1) * 512],
                    hs[:, n * 512:(n + 1) * 512], HS_C,
                    wo_ps[:], op0=ALU.mult, op1=ALU.add)

            # ================= AllReduce (bf16, x32 scaled) =============
            cc_in = dram.tile([B, DIM], BF)
            cc_out = dram.tile([B, DIM], BF)
            nc.gpsimd.dma_start(cc_in[:], wo_out[:])

            nc.gpsimd.collective_compute(
                "AllReduce", ALU.add,
                replica_groups=[list(range(NCORES))],
                ins=[cc_in[:].opt()], outs=[cc_out[:].opt()],
            )

            # ar = hidden * WO_SCALE (residual already included); reuses
            # wo_out's buffer (free after the cc_in copy)
            ar = sb.tile([B, DIM], BF, tag="wo_out")
            nc.gpsimd.dma_start(ar[:], cc_out[:])

            # hT directly from ar (stays in x32 wire units, bf16)
            hT = sb.tile([128, B * DIM // 128], BF, tag="hT")
            transpose_rows(ar, DIM, hT, id64b)

            # hidden (true units) for res2 + rmsnorm; off the critical path
            hidden = sb.tile([B, DIM], BF, tag="hid")
            nc.vector.tensor_scalar_mul(hidden[:], ar[:], 1.0 / WO_SCALE)
            nc.gpsimd.dma_start(res2_d[:], hidden[:])

            # ========== RMSNorm 2 (deferred, wire-unscale folded) =======
            # rstd_wire = rstd_true/WO_SCALE: fold via
            # t1 = ssq*(WO_SCALE^2/DIM) + WO_SCALE^2*eps
            rstd2h = rmsnorm_rstd(hidden, "n2",
                                  c0=WO_SCALE * WO_SCALE / DIM,
                                  c1=WO_SCALE * WO_SCALE * EPS)

            ug_slabs = {}

            def ug_block(idx):
                s = SLAB_UG + idx // 9
                if s not in ug_slabs:
                    ug_slabs[s] = slab(s)
                return ug_slabs[s], (idx % 9) * CW

            g_row = sb.tile([B, IL], BF, tag="g_row")
            gu_row = sb.tile([B, IL], BF, tag="gu_row")
            for c in range(4):
                up_ps = ps_acc.tile([B, CW], FP, tag="acc")
                gt_ps = ps_acc.tile([B, CW], FP, tag="acc")
                for j in range(32):
                    su, cu = ug_block(c * 64 + j * 2)
                    sg, cg = ug_block(c * 64 + j * 2 + 1)
                    nc.tensor.matmul(up_ps[:],
                                     hT[:, j * 64:(j + 1) * 64],
                                     su[:].bitcast(BF)[:, cu:cu + CW],
                                     start=(j == 0), stop=(j == 31))
                    nc.tensor.matmul(gt_ps[:],
                                     hT[:, j * 64:(j + 1) * 64],
                                     sg[:].bitcast(BF)[:, cg:cg + CW],
                                     start=(j == 0), stop=(j == 31))
                # g = silu(rstd_w * gate_wire); gu = (up_wire*rstd_w) * g
                nc.scalar.activation(g_row[:, c * CW:(c + 1) * CW], gt_ps[:],
                                     AF.Silu, scale=rstd2h[:])
                nc.vector.scalar_tensor_tensor(
                    gu_row[:, c * CW:(c + 1) * CW], up_ps[:], rstd2h[:],
                    g_row[:, c * CW:(c + 1) * CW],
                    op0=ALU.mult, op1=ALU.mult)

            guT = sb.tile([128, 14 * 64], BF, tag="guT")
            transpose_rows(gu_row, IL, guT, id64b)

            dn_slabs = {}

            def dn_block(idx):
                s = SLAB_DN + idx // 8
                if s not in dn_slabs:
                    dn_slabs[s] = slab(s)
                return dn_slabs[s], (idx % 8) * 512

            for n in range(8):
                dn_ps = ps_acc.tile([B, 512], FP, tag="acc")
                for cc in range(14):
                    sd, col = dn_block(n * 14 + cc)
                    nc.tensor.matmul(dn_ps[:], guT[:, cc * 64:(cc + 1) * 64],
                                     sd[:].bitcast(BF)[:, col:col + 512],
                                     start=(cc == 0), stop=(cc == 13))
                stg = small.tile([B, 512], BF, tag="ostg", bufs=2)
                nc.vector.tensor_copy(stg[:], dn_ps[:])
                nc.gpsimd.dma_start(partial_d[:, n * 512:(n + 1) * 512],
                                    stg[:])

            assert next_slab[0] == NSLAB, next_slab[0]

    nc.compile()
    return nc


def shard_inputs(inputs):
    """Full fp32 inputs -> list of 8 per-core input maps (host prep)."""
    f32 = np.float32
    bf16 = mybir.dt.np(BF)
    f8 = mybir.dt.np(F8)
    hs = np.ascontiguousarray(inputs["hidden_states"].reshape(B, DIM), f32)
    wqkv = np.asarray(inputs["wqkv_w"], f32)
    wb = np.asarray(inputs["wqkv_b"], f32)
    wo = np.asarray(inputs["wo_w"], f32)
    up = np.asarray(inputs["up_w"], f32)
    gate = np.asarray(inputs["gate_w"], f32)
    down = np.asarray(inputs["down_w"], f32)
    qnorm = np.asarray(inputs["qnorm_w"], f32)
    knorm = np.asarray(inputs["knorm_w"], f32)
    iln = np.asarray(inputs["in_ln_w"], f32)
    pln = np.asarray(inputs["post_ln_w"], f32)
    kc = np.asarray(inputs["k_cache"], f32)   # [B, S, 8, HD]
    vc = np.asarray(inputs["v_cache"], f32)

    # selection matrix [128 bands, 16]: SEL[32b+g, 4b+g] = 1
    sel = np.zeros((HD, 16), f8)
    mask4 = np.zeros((GRP, HD), f32)
    for b in range(GRP):
        for g in range(G):
            sel[32 * b + g, 4 * b + g] = 1.0
            mask4[b, 32 * b + g] = 1.0

    H = 32
    maps = []
    for c in range(NCORES):
        strm = np.zeros((NSLAB, HD, SLAB_W), f8)

        # --- slabs 0-3: wqkvT images (8 j-blocks of 768 cols each) ---
        wq = wqkv[c * G * HD:(c + 1) * G * HD]              # [512, DIM]
        wk = wqkv[H * HD + c * HD:H * HD + (c + 1) * HD]    # [128, DIM]
        wv = wqkv[(H + 8) * HD + c * HD:(H + 8) * HD + (c + 1) * HD]
        wloc = np.concatenate([wq, wk, wv], axis=0)         # [768, DIM]
        wqkvT = (wloc * iln[None, :] * WQ_SCALE).T.astype(f8)  # [DIM, 768]
        strm[0:4, :, 0:8 * QKV] = (
            wqkvT.reshape(4, 8, HD, QKV).transpose(0, 2, 1, 3)
            .reshape(4, HD, 8 * QKV))

        bq = wb[c * G * HD:(c + 1) * G * HD]
        bk = wb[H * HD + c * HD:H * HD + (c + 1) * HD]
        bv = wb[(H + 8) * HD + c * HD:(H + 8) * HD + (c + 1) * HD]
        biasc = np.ascontiguousarray(
            np.concatenate([bq, bk, bv]).reshape(6, HD).T)  # [128, 6]

        # --- packed constant image ---
        cst8 = np.zeros((128, CST_W), np.uint8)
        cst8[:, C_ID128:C_ID128 + 128] = np.eye(128, dtype=f8).view(np.uint8)
        cst8[:, C_SEL:C_SEL + 16] = sel.view(np.uint8)
        cst8[:, C_ONES128:C_ONES128 + 4] = \
            np.ones((HD, 1), f32).view(np.uint8)
        cst8[:, C_BIASC:C_BIASC + 24] = biasc.astype(f32).view(np.uint8)
        cst8[0:64, C_ID64Q:C_ID64Q + 64] = \
            np.eye(64, dtype=f8).view(np.uint8)
        cst8[0:64, C_ID64B:C_ID64B + 128] = \
            np.eye(64, dtype=bf16).view(np.uint8)
        cst8[0:1, C_ONES14:C_ONES14 + 4] = \
            np.ones((1, GRP), f8).view(np.uint8)
        cst8[0:1, C_QNW:C_QNW + 512] = \
            qnorm.reshape(1, HD).astype(f32).view(np.uint8)
        cst8[0:1, C_KNW:C_KNW + 512] = \
            knorm.reshape(1, HD).astype(f32).view(np.uint8)
        cst8[0:4, C_MASK4:C_MASK4 + 512] = mask4.view(np.uint8)
        cst = cst8.view(f8)

        # --- slabs 4..35: KV (k seq-transposed, v seq-major) ---
        kT = kc[:, :, c, :].transpose(0, 2, 1).astype(f8)   # [B, HD, S]
        vsm = (vc[:, :, c, :].reshape(B, 16, 128, HD)
               .transpose(0, 2, 1, 3).reshape(B, HD, S).astype(f8))
        for t in range(NGRP):
            for b in range(GRP):
                strm[SLAB_KV0 + 2 * t, :, b * S:(b + 1) * S] = kT[t * GRP + b]
                strm[SLAB_KV0 + 2 * t + 1, :, b * S:(b + 1) * S] = \
                    vsm[t * GRP + b]

        # --- slabs 36-37: woT ---
        woT = (wo[:, c * G * HD:(c + 1) * G * HD].T * WO_SCALE).astype(f8)
        wo_img = (woT.reshape(4, HD, DIM).transpose(1, 0, 2)
                  .reshape(HD, 4 * DIM))
        strm[SLAB_WO] = wo_img[:, 0:SLAB_W]
        strm[SLAB_WO + 1] = wo_img[:, SLAB_W:2 * SLAB_W]

        # --- slabs 38..66: up/gate bf16, block id = c*64+j*2+{0=up,1=gate},
        #     9 blocks of 448 bf16 cols per slab ---
        upT = ((up[c * IL:(c + 1) * IL] * pln[None, :]).T).astype(bf16)
        gateT = ((gate[c * IL:(c + 1) * IL] * pln[None, :]).T).astype(bf16)
        ug_bf = np.zeros((NSLAB_UG, HD, SLAB_W // 2), bf16)
        for cch in range(4):
            for j in range(32):
                for g, wT in ((0, upT), (1, gateT)):
                    idx = cch * 64 + j * 2 + g
                    s_, b_ = idx // 9, (idx % 9) * CW
                    ug_bf[s_, :, b_:b_ + CW] = \
                        wT[j * HD:(j + 1) * HD, cch * CW:(cch + 1) * CW]
        strm[SLAB_UG:SLAB_UG + NSLAB_UG] = \
            ug_bf.view(np.uint8).reshape(NSLAB_UG, HD, SLAB_W).view(f8)

        # --- slabs 67..80: down bf16, block id = n*14+cc, 8 of 512/slab ---
        downT = down[:, c * IL:(c + 1) * IL].T.astype(bf16)  # [IL, DIM]
        dn_bf = np.zeros((NSLAB_DN, HD, SLAB_W // 2), bf16)
        for n in range(8):
            for cc in range(14):
                idx = n * 14 + cc
                s_, b_ = idx // 8, (idx % 8) * 512
                dn_bf[s_, :, b_:b_ + 512] = \
                    downT[cc * HD:(cc + 1) * HD, n * 512:(n + 1) * 512]
        strm[SLAB_DN:SLAB_DN + NSLAB_DN] = \
            dn_bf.view(np.uint8).reshape(NSLAB_DN, HD, SLAB_W).view(f8)

        maps.append({"strm": strm, "hs": hs, "cst": cst})
    return maps


_NC = None


def _get_nc():
    global _NC
    if _NC is None:
        _NC = build_nc()
    return _NC


def run(inputs, **kw):
    nc = _get_nc()
    in_maps = shard_inputs(inputs)
    res = run_bass_kernel_spmd(nc, in_maps, list(range(NCORES)), **kw)
    out = res.results[0]["res2"].astype(np.float64)
    for c in range(NCORES):
        out = out + res.results[c]["partial"].astype(np.float64)
    return out.astype(np.float32).reshape(B, 1, DIM), res


def kernel(**inputs):
    out, _ = run(inputs)
    return out
